# revision 2
# baseline (speedup 1.0000x reference)
"""Trainium2 Bass kernel for DimeNet-style Interaction block (gnn_message_passing).

Strategy (8 NeuronCores, no collectives):
  - Host: sort triplets by edge_index_to; split edges into 8 equal contiguous
    slices (one per core). Each core gets its triplet run, grouped into blocks
    of 256 triplets (2 subtiles of 128) that cover <= 128 consecutive edges
    (cut at run boundaries; pad ~1.5%). Host pre-gathers x/radial rows into
    triplet order (transposed layouts), so the device program is fully dense:
    no indirect DMA at all.
  - Device per core:
      x_kj^T = silu(w_from^T @ x_g^T + b) * (w_rbf^T @ radial_g^T)  (fp32r MMs)
      per 128-triplet subtile:
        sbf   = sph_tile^T @ w_sbf                       [128,8]   (fp32 MM)
        tmp   = x_kj_tile^T.T @ W2                       [128,1024] (fp32r MMs)
        tmp'j = tmp_j * sbf[:,j]  (ACT/DVE scale-copy, PSUM->SBUF, bf16)
        S     = (iota == to_local)  (DVE is_equal, bf16)
        agg  += S^T @ tmp'_j  (8 bf16 MMs, PSUM-accumulated per block)
      drain agg -> transpose on PE -> slot-layout agg^T [128, NB*128] in SBUF
      epilogue on slot columns (transposed layout, fp32r MMs + Silu ACT):
        h = silu(x@w_to+b) + agg ; residual(rb) ; h = silu(h@lin+b)+x ; 2x residual(ra)
  - Host: compact slot columns -> edge rows, concat cores.
"""
import os
import numpy as np

H, B, NR, NS = 128, 8, 6, 7
P = 128
BLK_T = 256          # triplets per block (2 subtiles of 128)
SLOT_W = 128         # slot width; block edge-coverage <= SLOT_W
N_CORES = 8
EP_N = 512           # epilogue column-block width


def _silu_np(x):
    return x / (1.0 + np.exp(-x))


def host_prep(x, radial, sph, e_from, e_to):
    E_ = x.shape[0]
    perm = np.argsort(e_to, kind='stable')
    to_s = e_to[perm].astype(np.int64)
    from_s = e_from[perm].astype(np.int64)

    edges_per_core = (E_ + N_CORES - 1) // N_CORES
    bounds = np.searchsorted(to_s, [c * edges_per_core for c in range(N_CORES + 1)])

    cores = []
    for c in range(N_CORES):
        t0, t1 = bounds[c], bounds[c + 1]
        e_lo = c * edges_per_core
        e_hi = min((c + 1) * edges_per_core, E_)
        ct = to_s[t0:t1] - e_lo
        cf = from_s[t0:t1]
        psl = perm[t0:t1]
        n = len(ct)
        blocks = []
        i = 0
        cov_lo = 0
        while i < n:
            start = i
            j = i
            while j < n:
                v = ct[j]
                k = j
                while k < n and ct[k] == v:
                    k += 1
                if v - cov_lo >= SLOT_W:
                    break
                if (k - start) > BLK_T:
                    break
                j = k
            if j == start:
                blocks.append((start, start, cov_lo))
                cov_lo = int(ct[start])
                continue
            blocks.append((start, j, cov_lo))
            cov_lo = int(ct[j - 1]) + 1
            i = j
        local_end = e_hi - e_lo
        while cov_lo < local_end:
            blocks.append((n, n, cov_lo))
            cov_lo = min(cov_lo + SLOT_W, local_end)
        cores.append(dict(e_lo=e_lo, e_hi=e_hi, ct=ct, cf=cf, psl=psl, blocks=blocks))

    NB = max(len(c['blocks']) for c in cores)
    NB = max(NB, 2)
    if NB % 2:
        NB += 1   # keep W_S a multiple of 256
    T_pad = NB * BLK_T
    W_S = NB * SLOT_W

    for core in cores:
        blocks = core['blocks']
        ct, cf, psl = core['ct'], core['cf'], core['psl']
        e_lo, e_hi = core['e_lo'], core['e_hi']
        while len(blocks) < NB:
            blocks.append((len(ct), len(ct), e_hi - e_lo))
        xg = np.zeros((T_pad, H), np.float32)
        radg = np.zeros((T_pad, NR), np.float32)
        sphg = np.zeros((T_pad, NS * NR), np.float32)
        tol = np.zeros((T_pad,), np.float32)
        cov_lo_arr = np.zeros((NB,), np.int64)
        cov_w_arr = np.zeros((NB,), np.int64)
        for b, (ts_, te_, cov_lo) in enumerate(blocks):
            cnt = te_ - ts_
            dst = b * BLK_T
            if cnt:
                xg[dst:dst + cnt] = x[cf[ts_:te_]]
                radg[dst:dst + cnt] = radial[cf[ts_:te_]]
                sphg[dst:dst + cnt] = sph[psl[ts_:te_]]
                tol[dst:dst + cnt] = (ct[ts_:te_] - cov_lo).astype(np.float32)
            cov_lo_arr[b] = cov_lo
            nxt = blocks[b + 1][2] if b + 1 < len(blocks) else (e_hi - e_lo)
            cov_w_arr[b] = max(0, min(nxt, e_hi - e_lo) - cov_lo)
        x_slots = np.zeros((W_S, H), np.float32)
        for b in range(NB):
            lo, w = int(cov_lo_arr[b]), int(cov_w_arr[b])
            if w > 0:
                x_slots[b * SLOT_W: b * SLOT_W + w] = x[e_lo + lo: e_lo + lo + w]
        core['xg_T'] = np.ascontiguousarray(xg.T)
        core['radg_T'] = np.ascontiguousarray(radg.T)
        core['sph_T'] = np.ascontiguousarray(sphg.T)
        # to_local packed as one column per subtile: [128, 2*NB]
        core['tol_cols'] = np.ascontiguousarray(tol.reshape(2 * NB, P).T)
        core['x_slots_T'] = np.ascontiguousarray(x_slots.T)
        core['cov_lo'] = cov_lo_arr
        core['cov_w'] = cov_w_arr
    return cores, dict(NB=NB, T_pad=T_pad, W_S=W_S, edges_per_core=edges_per_core)


def build_program(NB, T_pad, W_S):
    import concourse.bass as bass
    import concourse.tile as tile
    from concourse import bacc, mybir

    f32 = mybir.dt.float32
    f32r = mybir.dt.float32r
    bf16 = mybir.dt.bfloat16
    AF = mybir.ActivationFunctionType
    ALU = mybir.AluOpType

    nc = bacc.Bacc(None, target_bir_lowering=False)
    # inputs
    xg_T = nc.dram_tensor("xg_T", [P, T_pad], f32, kind="ExternalInput")
    radg_T = nc.dram_tensor("radg_T", [NR, T_pad], f32, kind="ExternalInput")
    sph_T = nc.dram_tensor("sph_T", [NS * NR, T_pad], f32, kind="ExternalInput")
    x_slots_T = nc.dram_tensor("x_slots_T", [P, W_S], f32, kind="ExternalInput")
    MW = 2 * P + 9 + B + 2 * NB
    cmisc_d = nc.dram_tensor("cmisc", [P, MW], f32, kind="ExternalInput")
    CW = H + B * H + 8 * H + H
    cw_d = nc.dram_tensor("cw", [P, CW], f32, kind="ExternalInput")
    out_T = nc.dram_tensor("out_T", [P, W_S], f32, kind="ExternalOutput")

    with tile.TileContext(nc) as tc:
        with (
            tc.tile_pool(name="consts", bufs=1) as cp,
            tc.tile_pool(name="persist", bufs=1) as pp,
        ):
            cmisc_t = cp.tile([P, MW], f32)
            nc.gpsimd.dma_start(out=cmisc_t[:], in_=cmisc_d[:, :])
            cw_t = cp.tile([P, CW], f32r)
            nc.gpsimd.dma_start(out=cw_t[:], in_=cw_d[:, :].bitcast(f32r))
            aggT_big = pp.tile([P, W_S], f32)

            iota_t = cmisc_t[:, 0:P]
            ident_t = cmisc_t[:, P:2 * P]
            bias_t = cmisc_t[:, 2 * P:2 * P + 9]
            w_sbf_t = cmisc_t[0:NS * NR, 2 * P + 9:2 * P + 9 + B]
            tol_t = cmisc_t[:, 2 * P + 9 + B:MW]
            w_from_t = cw_t[:, 0:H]
            W2_t = cw_t[:, H:H + B * H]
            epw_t = cw_t[:, H + B * H:H + B * H + 8 * H]
            w_rbf_t = cw_t[0:NR, H + B * H + 8 * H:H + B * H + 9 * H]
            b_from = bias_t[:, 0:1]

            # ---------------- main loop ----------------
            with (
                tc.tile_pool(name="mio", bufs=4) as mio,       # dma-in tiles
                tc.tile_pool(name="mwork", bufs=3) as mwork,   # sbuf work tiles
                tc.tile_pool(name="ptmp", bufs=1, space="PSUM") as ptmp,
                tc.tile_pool(name="pxk", bufs=1, space="PSUM") as pxk,
                tc.tile_pool(name="pagg", bufs=2, space="PSUM") as pagg,
                tc.tile_pool(name="psmall", bufs=1, space="PSUM") as psmall,
            ):
                for b in range(NB):
                    c0 = b * BLK_T
                    xg = mio.tile([P, BLK_T], f32r, tag="xg")
                    nc.gpsimd.dma_start(out=xg[:], in_=xg_T[:, c0:c0 + BLK_T].bitcast(f32r))
                    rad = mio.tile([NR, BLK_T], f32r, tag="rad")
                    nc.gpsimd.dma_start(out=rad[:], in_=radg_T[:, c0:c0 + BLK_T].bitcast(f32r))
                    sph = mio.tile([NS * NR, BLK_T], f32, tag="sph")
                    nc.gpsimd.dma_start(out=sph[:], in_=sph_T[:, c0:c0 + BLK_T])

                    xkj_p = pxk.tile([P, BLK_T], f32, tag="xkj_p")
                    nc.tensor.matmul(out=xkj_p[:], lhsT=w_from_t,
                                     rhs=xg[:], start=True, stop=True)
                    rbf_p = pxk.tile([P, BLK_T], f32, tag="rbf_p")
                    nc.tensor.matmul(out=rbf_p[:], lhsT=w_rbf_t,
                                     rhs=rad[:], start=True, stop=True)
                    xkj_s = mwork.tile([P, BLK_T], f32, tag="xkj_s")
                    nc.scalar.activation(out=xkj_s[:], in_=xkj_p[:], func=AF.Silu,
                                         bias=b_from, scale=1.0)
                    xkj = mwork.tile([P, BLK_T], f32r, tag="xkj")
                    nc.vector.tensor_tensor(out=xkj[:], in0=xkj_s[:], in1=rbf_p[:],
                                            op=ALU.mult)

                    agg_p = pagg.tile([P, P], f32, tag="agg")
                    for s in range(2):
                        w0 = s * P
                        sbf_p = psmall.tile([P, B], f32, tag="sbf_p")
                        nc.tensor.matmul(out=sbf_p[:], lhsT=sph[:, w0:w0 + P],
                                         rhs=w_sbf_t, start=True, stop=True)
                        sbf_s = mwork.tile([P, B], f32, tag="sbf_s")
                        nc.scalar.activation(out=sbf_s[:], in_=sbf_p[:], func=AF.Copy)

                        tmpA = ptmp.tile([P, 4 * H], f32, tag="tmpA")
                        nc.tensor.matmul(out=tmpA[:], lhsT=xkj[:, w0:w0 + P],
                                         rhs=W2_t[:, 0:4 * H],
                                         start=True, stop=True)
                        tmpB = ptmp.tile([P, 4 * H], f32, tag="tmpB")
                        nc.tensor.matmul(out=tmpB[:], lhsT=xkj[:, w0:w0 + P],
                                         rhs=W2_t[:, 4 * H:8 * H],
                                         start=True, stop=True)

                        S = mwork.tile([P, P], bf16, tag="S")
                        nc.vector.tensor_tensor(
                            out=S[:],
                            in0=tol_t[:, 2 * b + s: 2 * b + s + 1].to_broadcast([P, P]),
                            in1=iota_t, op=ALU.is_equal)
                        tmpS = mwork.tile([P, B * H], bf16, tag="tmpS")
                        for j in range(B):
                            src = tmpA[:, j * H:(j + 1) * H] if j < 4 else \
                                  tmpB[:, (j - 4) * H:(j - 3) * H]
                            dst = tmpS[:, j * H:(j + 1) * H]
                            sc = sbf_s[:, j:j + 1]
                            if j % 2 == 0:
                                nc.scalar.activation(out=dst, in_=src, func=AF.Copy,
                                                     scale=sc)
                            else:
                                nc.vector.tensor_tensor(
                                    out=dst, in0=src,
                                    in1=sc.to_broadcast([P, H]), op=ALU.mult)
                        for j in range(B):
                            nc.tensor.matmul(out=agg_p[:], lhsT=S[:],
                                             rhs=tmpS[:, j * H:(j + 1) * H],
                                             start=(s == 0 and j == 0),
                                             stop=(s == 1 and j == B - 1),
                                             skip_group_check=True)
                    agg_s = mwork.tile([P, P], f32, tag="agg_s")
                    nc.scalar.activation(out=agg_s[:], in_=agg_p[:], func=AF.Copy)
                    aggT_p = psmall.tile([P, P], f32, tag="aggT_p")
                    nc.tensor.transpose(out=aggT_p[:], in_=agg_s[:], identity=ident_t)
                    nc.vector.tensor_copy(out=aggT_big[:, b * SLOT_W:(b + 1) * SLOT_W],
                                          in_=aggT_p[:])

            # ---------------- epilogue ----------------
            with (
                tc.tile_pool(name="eio", bufs=3) as eio,
                tc.tile_pool(name="ework", bufs=2) as ework,
                tc.tile_pool(name="epsum", bufs=4, space="PSUM") as epsum,
            ):
                def ep_mm(lhs_idx, rhs_ap, n):
                    pt = epsum.tile([P, EP_N], f32, tag="ep_p")
                    nc.tensor.matmul(out=pt[:, :n],
                                     lhsT=epw_t[:, lhs_idx * H:(lhs_idx + 1) * H],
                                     rhs=rhs_ap, start=True, stop=True)
                    return pt

                def ep_silu(pt, bias_idx, n, tag, dt_=f32):
                    t = ework.tile([P, EP_N], dt_, tag=tag)
                    nc.scalar.activation(out=t[:, :n], in_=pt[:, :n], func=AF.Silu,
                                         bias=bias_t[:, bias_idx:bias_idx + 1], scale=1.0)
                    return t

                n_ep = (W_S + EP_N - 1) // EP_N
                for eb in range(n_ep):
                    c0 = eb * EP_N
                    n = min(EP_N, W_S - c0)
                    x_t = eio.tile([P, EP_N], f32r, tag="x_t")
                    nc.gpsimd.dma_start(out=x_t[:, :n], in_=x_slots_T[:, c0:c0 + n].bitcast(f32r))
                    # h = silu(x@w_to+b_to) + agg
                    pt = ep_mm(0, x_t[:, :n], n)
                    xji = ep_silu(pt, 1, n, "xji")
                    h = ework.tile([P, EP_N], f32r, tag="h")
                    nc.vector.tensor_tensor(out=h[:, :n], in0=xji[:, :n],
                                            in1=aggT_big[:, c0:c0 + n], op=ALU.add)
                    # rb residual
                    t1 = ep_silu(ep_mm(1, h[:, :n], n), 2, n, "t1", f32r)
                    t2 = ep_silu(ep_mm(2, t1[:, :n], n), 3, n, "t2")
                    h2 = ework.tile([P, EP_N], f32r, tag="h2")
                    nc.vector.tensor_tensor(out=h2[:, :n], in0=h[:, :n], in1=t2[:, :n],
                                            op=ALU.add)
                    # lin + skip x
                    l1 = ep_silu(ep_mm(3, h2[:, :n], n), 4, n, "l1")
                    h3 = ework.tile([P, EP_N], f32r, tag="h3")
                    nc.vector.tensor_tensor(out=h3[:, :n], in0=l1[:, :n], in1=x_t[:, :n],
                                            op=ALU.add)
                    # ra residuals x2
                    t3 = ep_silu(ep_mm(4, h3[:, :n], n), 5, n, "t3", f32r)
                    t4 = ep_silu(ep_mm(5, t3[:, :n], n), 6, n, "t4")
                    h4 = ework.tile([P, EP_N], f32r, tag="h4")
                    nc.vector.tensor_tensor(out=h4[:, :n], in0=h3[:, :n], in1=t4[:, :n],
                                            op=ALU.add)
                    t5 = ep_silu(ep_mm(6, h4[:, :n], n), 7, n, "t5", f32r)
                    t6 = ep_silu(ep_mm(7, t5[:, :n], n), 8, n, "t6")
                    h5 = ework.tile([P, EP_N], f32, tag="h5")
                    nc.vector.tensor_tensor(out=h5[:, :n], in0=h4[:, :n], in1=t6[:, :n],
                                            op=ALU.add)
                    nc.gpsimd.dma_start(out=out_T[:, c0:c0 + n], in_=h5[:, :n])
    nc.compile()
    return nc


def kernel(x, radial_basis, spherical_basis, edge_index_from, edge_index_to,
           w_rbf, w_sbf, w_from, b_from, w_to, b_to, W,
           rb_w, rb_b, lin_w, lin_b, ra_w, ra_b):
    from concourse.bass_utils import run_bass_kernel_spmd

    x = np.asarray(x, np.float32)
    radial = np.asarray(radial_basis, np.float32)
    sph = np.asarray(spherical_basis, np.float32)
    e_from = np.asarray(edge_index_from)
    e_to = np.asarray(edge_index_to)
    in_dtype = np.asarray(x).dtype

    cores, meta = host_prep(x, radial, sph, e_from, e_to)
    NB, T_pad, W_S = meta['NB'], meta['T_pad'], meta['W_S']

    W_np = np.asarray(W, np.float32)
    W2 = np.ascontiguousarray(W_np.transpose(2, 1, 0).reshape(H, B * H))
    ep_w = np.concatenate([
        np.asarray(w_to, np.float32),
        np.asarray(rb_w, np.float32)[0, 0], np.asarray(rb_w, np.float32)[0, 1],
        np.asarray(lin_w, np.float32),
        np.asarray(ra_w, np.float32)[0, 0], np.asarray(ra_w, np.float32)[0, 1],
        np.asarray(ra_w, np.float32)[1, 0], np.asarray(ra_w, np.float32)[1, 1],
    ], axis=1)
    biases = np.stack([
        np.asarray(b_from, np.float32), np.asarray(b_to, np.float32),
        np.asarray(rb_b, np.float32)[0, 0], np.asarray(rb_b, np.float32)[0, 1],
        np.asarray(lin_b, np.float32),
        np.asarray(ra_b, np.float32)[0, 0], np.asarray(ra_b, np.float32)[0, 1],
        np.asarray(ra_b, np.float32)[1, 0], np.asarray(ra_b, np.float32)[1, 1],
    ], axis=1).astype(np.float32)
    iota = np.tile(np.arange(P, dtype=np.float32), (P, 1))
    ident = np.eye(P, dtype=np.float32)

    # packed constants: cw (f32r-consumed weights) [P, 128+1024+1024+128]
    cw = np.zeros((P, H + B * H + 8 * H + H), np.float32)
    cw[:, 0:H] = np.asarray(w_from, np.float32)
    cw[:, H:H + B * H] = W2
    cw[:, H + B * H:H + B * H + 8 * H] = ep_w
    cw[0:NR, H + B * H + 8 * H:H + B * H + 9 * H] = np.asarray(w_rbf, np.float32)
    cw = np.ascontiguousarray(cw)

    nc = build_program(NB, T_pad, W_S)

    in_maps = []
    for core in cores:
        MW = 2 * P + 9 + B + 2 * NB
        cmisc = np.zeros((P, MW), np.float32)
        cmisc[:, 0:P] = iota
        cmisc[:, P:2 * P] = ident
        cmisc[:, 2 * P:2 * P + 9] = biases
        cmisc[0:NS * NR, 2 * P + 9:2 * P + 9 + B] = np.asarray(w_sbf, np.float32)
        cmisc[:, 2 * P + 9 + B:MW] = core['tol_cols']
        in_maps.append({
            "xg_T": core['xg_T'], "radg_T": core['radg_T'], "sph_T": core['sph_T'],
            "x_slots_T": core['x_slots_T'],
            "cmisc": np.ascontiguousarray(cmisc), "cw": cw,
        })
    res = run_bass_kernel_spmd(nc, in_maps, core_ids=list(range(N_CORES)))
    kernel._last_results = res
    if os.environ.get("KERNEL_EXEC_TWICE"):
        import time as _time
        os.environ["BASS_NEVER_TRACE"] = "1"
        try:
            t0 = _time.perf_counter()
            run_bass_kernel_spmd(nc, in_maps, core_ids=list(range(N_CORES)))
            kernel._exec2_s = _time.perf_counter() - t0
        finally:
            os.environ.pop("BASS_NEVER_TRACE", None)

    E_ = x.shape[0]
    out = np.zeros((E_, H), np.float32)
    for core, om in zip(cores, res.results):
        hT = om["out_T"]
        for b in range(NB):
            lo, w = int(core['cov_lo'][b]), int(core['cov_w'][b])
            if w > 0:
                out[core['e_lo'] + lo: core['e_lo'] + lo + w] = \
                    hT[:, b * SLOT_W: b * SLOT_W + w].T
    return out.astype(in_dtype, copy=False)



# revision 4
# speedup vs baseline: 31.7760x; 31.7760x over previous
"""Trainium2 Bass kernel for DimeNet-style Interaction block (gnn_message_passing).

Strategy (8 NeuronCores, SPMD, no collectives). The end-to-end metric is
dominated by the host<->device tunnel (~79 MB/s H2D, ~50 MB/s D2H), so the
design minimizes shipped bytes:
  - Host: sort triplets by edge_index_to; split edges into 8 equal contiguous
    slices (one per core). Each core gets its triplet run, grouped into blocks
    of <=384 triplets (3 subtiles of 128) covering <=128 consecutive edges.
    Host pre-gathers per-triplet inputs: x rows as int8 (one global scale,
    folded into w_from on the host), radial rows and sbf = spherical@w_sbf in
    bf16. The device program is fully dense - no indirect DMA.
  - Device per core (bf16 matmuls, fp32 PSUM):
      x_kj^T = silu(w_from'^T @ xg^T + b) * (w_rbf^T @ radial^T)
      per 128-triplet subtile:
        tmp   = x_kj_tile^T.T @ W2             [128,1024] PSUM
        tmp'j = tmp_j * sbf[:,j]               (ACT/DVE scale, bf16)
        S     = (iota == to_local)             (DVE is_equal, bf16)
        agg  += S^T @ tmp'_j                   (8 bf16 MMs, PSUM-accumulated)
      drain agg -> PE transpose -> slot-layout agg^T [128, NB*128] bf16
      epilogue on slot columns: h = silu(x@w_to+b)+agg; residual stack (bf16).
  - Output shipped bf16 [128, W_S] per core; host compacts slots -> edge rows.
"""
import os
import numpy as np
import ml_dtypes

BF16 = ml_dtypes.bfloat16

H, B, NR, NS = 128, 8, 6, 7
P = 128
NSUB = 3
BLK_T = NSUB * P     # triplets per block
SLOT_W = 128         # block edge-coverage <= SLOT_W
N_CORES = 8
EP_N = 512           # epilogue column-block width

_PROG_CACHE = {}


def _enable_jax_compile_cache():
    try:
        import jax
        jax.config.update("jax_compilation_cache_dir", "/tmp/jax_cache")
        jax.config.update("jax_persistent_cache_min_compile_time_secs", 0)
        jax.config.update("jax_persistent_cache_min_entry_size_bytes", 0)
    except Exception:
        pass


_enable_jax_compile_cache()


def make_blocks(ct, local_end):
    """Greedy blocks over sorted local to-indices ct: each block takes whole
    runs of equal ct while (value - cov_lo) < SLOT_W and count <= BLK_T."""
    n = len(ct)
    blocks = []
    cov_lo = 0
    if n:
        run_starts = np.flatnonzero(np.r_[True, ct[1:] != ct[:-1]])
        run_vals = ct[run_starts]
        run_ends = np.r_[run_starts[1:], n]
        nruns = len(run_vals)
        r = 0
        while r < nruns:
            v0 = int(run_vals[r])
            if v0 - cov_lo >= SLOT_W:
                ts = int(run_starts[r])
                blocks.append((ts, ts, cov_lo))
                cov_lo += SLOT_W
                continue
            start_t = int(run_starts[r])
            r_val = int(np.searchsorted(run_vals, cov_lo + SLOT_W, side="left"))
            r_cnt = int(np.searchsorted(run_ends, start_t + BLK_T, side="right"))
            r_next = max(min(r_val, r_cnt), r + 1)
            te = int(run_ends[r_next - 1])
            assert te - start_t <= BLK_T, "edge in-degree exceeds BLK_T"
            blocks.append((start_t, te, cov_lo))
            cov_lo = int(run_vals[r_next - 1]) + 1
            r = r_next
    while cov_lo < local_end:
        blocks.append((n, n, cov_lo))
        cov_lo = min(cov_lo + SLOT_W, local_end)
    return blocks


def host_prep(x, radial, sph, e_from, e_to, w_sbf, x_scale):
    E_ = x.shape[0]
    perm = np.argsort(e_to, kind='stable')
    to_s = e_to[perm].astype(np.int64)
    from_s = e_from[perm].astype(np.int64)

    epc = (E_ + N_CORES - 1) // N_CORES
    bounds = np.searchsorted(to_s, [c * epc for c in range(N_CORES + 1)])

    # global source arrays (converted once)
    xq = np.clip(np.rint(x * (127.0 / x_scale)), -127, 127).astype(np.int8)
    rad16 = radial.astype(BF16)
    sbf_all = (sph @ w_sbf).astype(BF16)          # [T, B]
    x16 = x.astype(BF16)

    cores = []
    for c in range(N_CORES):
        t0, t1 = bounds[c], bounds[c + 1]
        e_lo = c * epc
        e_hi = min((c + 1) * epc, E_)
        ct = to_s[t0:t1] - e_lo
        blocks = make_blocks(ct, e_hi - e_lo)
        cores.append(dict(e_lo=e_lo, e_hi=e_hi, ct=ct, cf=from_s[t0:t1],
                          psl=perm[t0:t1], blocks=blocks))

    NB = max(max(len(c['blocks']) for c in cores), 2)
    if NB % 2:
        NB += 1
    T_pad = NB * BLK_T
    W_S = NB * SLOT_W

    for core in cores:
        blocks = core['blocks']
        ct, cf, psl = core['ct'], core['cf'], core['psl']
        e_lo, e_hi = core['e_lo'], core['e_hi']
        local_end = e_hi - e_lo
        n = len(ct)
        while len(blocks) < NB:
            blocks.append((n, n, local_end))
        barr = np.asarray(blocks, np.int64).reshape(NB, 3)
        ts_a, te_a, cov_lo_arr = barr[:, 0], barr[:, 1], barr[:, 2]
        cnt_a = te_a - ts_a
        # nonempty blocks tile [0, n) contiguously -> src order is identity
        dst = np.repeat(BLK_T * np.arange(NB) - ts_a, cnt_a) + np.arange(n)

        xg8 = np.zeros((T_pad, H), np.int8)
        radg = np.zeros((T_pad, NR), BF16)
        sbfg = np.zeros((T_pad, B), BF16)
        tol = np.zeros((T_pad,), np.float32)
        xg8[dst] = xq[cf]
        radg[dst] = rad16[cf]
        sbfg[dst] = sbf_all[psl]
        tol[dst] = (ct - np.repeat(cov_lo_arr, cnt_a)).astype(np.float32)

        nxt = np.r_[cov_lo_arr[1:], local_end]
        cov_w_arr = np.maximum(0, np.minimum(nxt, local_end) - cov_lo_arr)

        x_slots = np.zeros((W_S, H), BF16)
        for b in range(NB):
            lo, w = int(cov_lo_arr[b]), int(cov_w_arr[b])
            if w > 0:
                x_slots[b * SLOT_W: b * SLOT_W + w] = x16[e_lo + lo: e_lo + lo + w]

        core['xg_T'] = np.ascontiguousarray(xg8.T)
        core['radg_T'] = np.ascontiguousarray(radg.T)
        # per-subtile sbf columns: [128, NSUB*NB*B]
        core['sbf_cols'] = np.ascontiguousarray(
            sbfg.reshape(NSUB * NB, P, B).transpose(1, 0, 2).reshape(P, NSUB * NB * B))
        core['tol_cols'] = np.ascontiguousarray(tol.reshape(NSUB * NB, P).T)
        core['x_slots_T'] = np.ascontiguousarray(x_slots.T)
        core['cov_lo'] = cov_lo_arr
        core['cov_w'] = cov_w_arr
    return cores, dict(NB=NB, T_pad=T_pad, W_S=W_S, epc=epc)


def build_program(NB, T_pad, W_S):
    import concourse.bass as bass
    import concourse.tile as tile
    from concourse import bacc, mybir

    f32 = mybir.dt.float32
    bf16 = mybir.dt.bfloat16
    i8 = mybir.dt.int8
    AF = mybir.ActivationFunctionType
    ALU = mybir.AluOpType

    nc = bacc.Bacc(None, target_bir_lowering=False)
    xg_d = nc.dram_tensor("xg_T", [P, T_pad], i8, kind="ExternalInput")
    radg_d = nc.dram_tensor("radg_T", [NR, T_pad], bf16, kind="ExternalInput")
    sbf_d = nc.dram_tensor("sbf_cols", [P, NSUB * NB * B], bf16,
                           kind="ExternalInput")
    x_slots_d = nc.dram_tensor("x_slots_T", [P, W_S], bf16, kind="ExternalInput")
    MW = 2 * P + 9 + NSUB * NB
    cmisc_d = nc.dram_tensor("cmisc", [P, MW], f32, kind="ExternalInput")
    CW = H + B * H + 8 * H + H
    cw_d = nc.dram_tensor("cw", [P, CW], bf16, kind="ExternalInput")
    out_d = nc.dram_tensor("out_T", [P, W_S], bf16, kind="ExternalOutput")

    with tile.TileContext(nc) as tc:
        with (
            tc.tile_pool(name="consts", bufs=1) as cp,
            tc.tile_pool(name="persist", bufs=1) as pp,
        ):
            cmisc_t = cp.tile([P, MW], f32)
            nc.gpsimd.dma_start(out=cmisc_t[:], in_=cmisc_d[:, :])
            cw_t = cp.tile([P, CW], bf16)
            nc.gpsimd.dma_start(out=cw_t[:], in_=cw_d[:, :])
            sbf16_t = cp.tile([P, NSUB * NB * B], bf16)
            nc.gpsimd.dma_start(out=sbf16_t[:], in_=sbf_d[:, :])
            x_sb = cp.tile([P, W_S], bf16)
            nc.gpsimd.dma_start(out=x_sb[:], in_=x_slots_d[:, :])
            sbf_f = cp.tile([P, NSUB * NB * B], f32)
            nc.vector.tensor_copy(out=sbf_f[:], in_=sbf16_t[:])
            aggT_big = pp.tile([P, W_S], bf16)

            iota_t = cmisc_t[:, 0:P]
            ident_t = cmisc_t[:, P:2 * P]
            bias_t = cmisc_t[:, 2 * P:2 * P + 9]
            tol_t = cmisc_t[:, 2 * P + 9:MW]
            w_from_t = cw_t[:, 0:H]
            W2_t = cw_t[:, H:H + B * H]
            epw_t = cw_t[:, H + B * H:H + B * H + 8 * H]
            w_rbf_t = cw_t[0:NR, H + B * H + 8 * H:CW]
            b_from = bias_t[:, 0:1]

            # ---------------- main loop ----------------
            with (
                tc.tile_pool(name="mio", bufs=4) as mio,
                tc.tile_pool(name="mwork", bufs=3) as mwork,
                tc.tile_pool(name="ptmp", bufs=1, space="PSUM") as ptmp,
                tc.tile_pool(name="pxk", bufs=1, space="PSUM") as pxk,
                tc.tile_pool(name="pagg", bufs=2, space="PSUM") as pagg,
                tc.tile_pool(name="psmall", bufs=1, space="PSUM") as psmall,
            ):
                for b in range(NB):
                    c0 = b * BLK_T
                    xg8 = mio.tile([P, BLK_T], i8, tag="xg8")
                    nc.gpsimd.dma_start(out=xg8[:], in_=xg_d[:, c0:c0 + BLK_T])
                    rad = mio.tile([NR, BLK_T], bf16, tag="rad")
                    nc.gpsimd.dma_start(out=rad[:], in_=radg_d[:, c0:c0 + BLK_T])
                    xgc = mwork.tile([P, BLK_T], bf16, tag="xgc")
                    nc.vector.tensor_copy(out=xgc[:], in_=xg8[:])

                    xkj_p = pxk.tile([P, BLK_T], f32, tag="xkj_p")
                    nc.tensor.matmul(out=xkj_p[:], lhsT=w_from_t, rhs=xgc[:],
                                     start=True, stop=True)
                    rbf_p = pxk.tile([P, BLK_T], f32, tag="rbf_p")
                    nc.tensor.matmul(out=rbf_p[:], lhsT=w_rbf_t, rhs=rad[:],
                                     start=True, stop=True)
                    xkj_s = mwork.tile([P, BLK_T], f32, tag="xkj_s")
                    nc.scalar.activation(out=xkj_s[:], in_=xkj_p[:], func=AF.Silu,
                                         bias=b_from, scale=1.0)
                    xkj = mwork.tile([P, BLK_T], bf16, tag="xkj")
                    nc.vector.tensor_tensor(out=xkj[:], in0=xkj_s[:], in1=rbf_p[:],
                                            op=ALU.mult)

                    agg_p = pagg.tile([P, P], f32, tag="agg")
                    for s in range(NSUB):
                        w0 = s * P
                        sc0 = (NSUB * b + s) * B
                        tmpA = ptmp.tile([P, 4 * H], f32, tag="tmpA")
                        nc.tensor.matmul(out=tmpA[:], lhsT=xkj[:, w0:w0 + P],
                                         rhs=W2_t[:, 0:4 * H], start=True, stop=True)
                        tmpB = ptmp.tile([P, 4 * H], f32, tag="tmpB")
                        nc.tensor.matmul(out=tmpB[:], lhsT=xkj[:, w0:w0 + P],
                                         rhs=W2_t[:, 4 * H:8 * H], start=True,
                                         stop=True)

                        S = mwork.tile([P, P], bf16, tag="S")
                        nc.vector.tensor_tensor(
                            out=S[:],
                            in0=tol_t[:, NSUB * b + s: NSUB * b + s + 1]
                                .to_broadcast([P, P]),
                            in1=iota_t, op=ALU.is_equal)
                        tmpS = mwork.tile([P, B * H], bf16, tag="tmpS")
                        for j in range(B):
                            src = tmpA[:, j * H:(j + 1) * H] if j < 4 else \
                                  tmpB[:, (j - 4) * H:(j - 3) * H]
                            dst = tmpS[:, j * H:(j + 1) * H]
                            sc = sbf_f[:, sc0 + j:sc0 + j + 1]
                            if j % 2 == 0:
                                nc.scalar.activation(out=dst, in_=src, func=AF.Copy,
                                                     scale=sc)
                            else:
                                nc.vector.tensor_tensor(
                                    out=dst, in0=src,
                                    in1=sc.to_broadcast([P, H]), op=ALU.mult)
                        for j in range(B):
                            nc.tensor.matmul(out=agg_p[:], lhsT=S[:],
                                             rhs=tmpS[:, j * H:(j + 1) * H],
                                             start=(s == 0 and j == 0),
                                             stop=(s == NSUB - 1 and j == B - 1),
                                             skip_group_check=True)
                    agg_s = mwork.tile([P, P], f32, tag="agg_s")
                    nc.scalar.activation(out=agg_s[:], in_=agg_p[:], func=AF.Copy)
                    aggT_p = psmall.tile([P, P], f32, tag="aggT_p")
                    nc.tensor.transpose(out=aggT_p[:], in_=agg_s[:],
                                        identity=ident_t)
                    nc.vector.tensor_copy(
                        out=aggT_big[:, b * SLOT_W:(b + 1) * SLOT_W],
                        in_=aggT_p[:])

            # ---------------- epilogue ----------------
            with (
                tc.tile_pool(name="ework", bufs=2) as ework,
                tc.tile_pool(name="epsum", bufs=4, space="PSUM") as epsum,
            ):
                def ep_mm(lhs_idx, rhs_ap, n):
                    pt = epsum.tile([P, EP_N], f32, tag="ep_p")
                    nc.tensor.matmul(out=pt[:, :n],
                                     lhsT=epw_t[:, lhs_idx * H:(lhs_idx + 1) * H],
                                     rhs=rhs_ap, start=True, stop=True)
                    return pt

                def ep_silu(pt, bias_idx, n, tag):
                    t = ework.tile([P, EP_N], bf16, tag=tag)
                    nc.scalar.activation(out=t[:, :n], in_=pt[:, :n], func=AF.Silu,
                                         bias=bias_t[:, bias_idx:bias_idx + 1],
                                         scale=1.0)
                    return t

                def ep_add(a, b_, n, tag):
                    t = ework.tile([P, EP_N], bf16, tag=tag)
                    nc.vector.tensor_tensor(out=t[:, :n], in0=a, in1=b_, op=ALU.add)
                    return t

                n_ep = (W_S + EP_N - 1) // EP_N
                for eb in range(n_ep):
                    c0 = eb * EP_N
                    n = min(EP_N, W_S - c0)
                    x_sl = x_sb[:, c0:c0 + n]
                    xji = ep_silu(ep_mm(0, x_sl, n), 1, n, "xji")
                    h = ep_add(xji[:, :n], aggT_big[:, c0:c0 + n], n, "h")
                    t1 = ep_silu(ep_mm(1, h[:, :n], n), 2, n, "t1")
                    t2 = ep_silu(ep_mm(2, t1[:, :n], n), 3, n, "t2")
                    h2 = ep_add(h[:, :n], t2[:, :n], n, "h2")
                    l1 = ep_silu(ep_mm(3, h2[:, :n], n), 4, n, "l1")
                    h3 = ep_add(l1[:, :n], x_sl, n, "h3")
                    t3 = ep_silu(ep_mm(4, h3[:, :n], n), 5, n, "t3")
                    t4 = ep_silu(ep_mm(5, t3[:, :n], n), 6, n, "t4")
                    h4 = ep_add(h3[:, :n], t4[:, :n], n, "h4")
                    t5 = ep_silu(ep_mm(6, h4[:, :n], n), 7, n, "t5")
                    t6 = ep_silu(ep_mm(7, t5[:, :n], n), 8, n, "t6")
                    h5 = ep_add(h4[:, :n], t6[:, :n], n, "h5")
                    nc.gpsimd.dma_start(out=out_d[:, c0:c0 + n], in_=h5[:, :n])
    nc.compile()
    return nc


def kernel(x, radial_basis, spherical_basis, edge_index_from, edge_index_to,
           w_rbf, w_sbf, w_from, b_from, w_to, b_to, W,
           rb_w, rb_b, lin_w, lin_b, ra_w, ra_b):
    from concourse.bass_utils import run_bass_kernel_spmd

    x = np.asarray(x, np.float32)
    radial = np.asarray(radial_basis, np.float32)
    sph = np.asarray(spherical_basis, np.float32)
    e_from = np.asarray(edge_index_from)
    e_to = np.asarray(edge_index_to)
    in_dtype = np.asarray(x).dtype

    x_scale = float(np.abs(x).max()) or 1.0
    cores, meta = host_prep(x, radial, sph, e_from, e_to,
                            np.asarray(w_sbf, np.float32), x_scale)
    NB, T_pad, W_S = meta['NB'], meta['T_pad'], meta['W_S']

    W_np = np.asarray(W, np.float32)
    W2 = np.ascontiguousarray(W_np.transpose(2, 1, 0).reshape(H, B * H))
    ep_w = np.concatenate([
        np.asarray(w_to, np.float32),
        np.asarray(rb_w, np.float32)[0, 0], np.asarray(rb_w, np.float32)[0, 1],
        np.asarray(lin_w, np.float32),
        np.asarray(ra_w, np.float32)[0, 0], np.asarray(ra_w, np.float32)[0, 1],
        np.asarray(ra_w, np.float32)[1, 0], np.asarray(ra_w, np.float32)[1, 1],
    ], axis=1)
    biases = np.stack([
        np.asarray(b_from, np.float32), np.asarray(b_to, np.float32),
        np.asarray(rb_b, np.float32)[0, 0], np.asarray(rb_b, np.float32)[0, 1],
        np.asarray(lin_b, np.float32),
        np.asarray(ra_b, np.float32)[0, 0], np.asarray(ra_b, np.float32)[0, 1],
        np.asarray(ra_b, np.float32)[1, 0], np.asarray(ra_b, np.float32)[1, 1],
    ], axis=1).astype(np.float32)
    iota = np.tile(np.arange(P, dtype=np.float32), (P, 1))
    ident = np.eye(P, dtype=np.float32)

    CW = H + B * H + 8 * H + H
    cw = np.zeros((P, CW), np.float32)
    cw[:, 0:H] = np.asarray(w_from, np.float32) * (x_scale / 127.0)
    cw[:, H:H + B * H] = W2
    cw[:, H + B * H:H + B * H + 8 * H] = ep_w
    cw[0:NR, H + B * H + 8 * H:CW] = np.asarray(w_rbf, np.float32)
    cw16 = np.ascontiguousarray(cw.astype(BF16))

    key = (NB, T_pad, W_S)
    nc = _PROG_CACHE.get(key)
    if nc is None:
        nc = build_program(NB, T_pad, W_S)
        _PROG_CACHE[key] = nc

    MW = 2 * P + 9 + NSUB * NB
    in_maps = []
    for core in cores:
        cmisc = np.zeros((P, MW), np.float32)
        cmisc[:, 0:P] = iota
        cmisc[:, P:2 * P] = ident
        cmisc[:, 2 * P:2 * P + 9] = biases
        cmisc[:, 2 * P + 9:MW] = core['tol_cols']
        in_maps.append({
            "xg_T": core['xg_T'], "radg_T": core['radg_T'],
            "sbf_cols": core['sbf_cols'], "x_slots_T": core['x_slots_T'],
            "cmisc": np.ascontiguousarray(cmisc), "cw": cw16,
        })
    res = run_bass_kernel_spmd(nc, in_maps, core_ids=list(range(N_CORES)))
    kernel._last_results = res
    if os.environ.get("KERNEL_EXEC_TWICE"):
        import time as _time
        os.environ["BASS_NEVER_TRACE"] = "1"
        try:
            t0 = _time.perf_counter()
            run_bass_kernel_spmd(nc, in_maps, core_ids=list(range(N_CORES)))
            kernel._exec2_s = _time.perf_counter() - t0
        finally:
            os.environ.pop("BASS_NEVER_TRACE", None)

    E_ = x.shape[0]
    out = np.zeros((E_, H), np.float32)
    for core, om in zip(cores, res.results):
        hT = np.asarray(om["out_T"], dtype=np.float32)
        for b in range(NB):
            lo, w = int(core['cov_lo'][b]), int(core['cov_w'][b])
            if w > 0:
                out[core['e_lo'] + lo: core['e_lo'] + lo + w] = \
                    hT[:, b * SLOT_W: b * SLOT_W + w].T
    return out.astype(in_dtype, copy=False)
